# revision 26
# baseline (speedup 1.0000x reference)
import sys

if "/opt/trn_rl_repo" not in sys.path:
    sys.path.insert(0, "/opt/trn_rl_repo")

import numpy as np

import concourse.bass as bass
import concourse.tile as tile
from concourse import bacc, mybir
from concourse.bass_utils import run_bass_kernel_spmd
from concourse.masks import make_identity, make_upper_triangular

F32 = mybir.dt.float32
F32R = mybir.dt.float32r
BF16 = mybir.dt.bfloat16
FP16 = mybir.dt.float16

# Problem shape (hardcoded per contract)
B, T, D = 4, 2048, 768
H, HD = 12, 64
N_CORES = 8
HEADS_PER_CORE = 6          # 12 heads / 2 groups
CPC = HEADS_PER_CORE * HD   # 384 qkv columns per core
TC = T // 128               # 16 token tiles of 128
DC = D // 128               # 6 chunks of the model dim
CC = CPC // 128             # 3 chunks of this core's head cols
OC = D // 128               # 6 output-col chunks
WT = 512                    # wide tile / matmul free-dim limit
TW = T // WT                # 4 wide token tiles

# ragged P^T layout: block jc covers i in [jc*128, T); OFF[jc] is its col offset
OFF = [0] * (TC + 1)
for _jc in range(TC):
    OFF[_jc + 1] = OFF[_jc] + (TC - _jc) * 128
PT_COLS = OFF[TC]  # 17408

_CACHE = {}


def _build_nc():
    nc = bacc.Bacc("TRN2", target_bir_lowering=False, debug=False)

    xT = nc.dram_tensor("xT", [D, T], F32R, kind="ExternalInput")
    wq = nc.dram_tensor("wq", [D, CPC], F32R, kind="ExternalInput")
    wk = nc.dram_tensor("wk", [D, CPC], F32R, kind="ExternalInput")
    wv = nc.dram_tensor("wv", [D, CPC], F32R, kind="ExternalInput")
    bq = nc.dram_tensor("bq", [128, CC], F32, kind="ExternalInput")
    bk = nc.dram_tensor("bk", [128, CC], F32, kind="ExternalInput")
    bv = nc.dram_tensor("bv", [128, CPC], F32, kind="ExternalInput")
    wo = nc.dram_tensor("wo", [CPC, D], F32R, kind="ExternalInput")
    yT = nc.dram_tensor("yT", [D, T], F32, kind="ExternalOutput")

    with tile.TileContext(nc) as tc:
        with tc.tile_pool(name="persist", bufs=1) as pp:
            qT_sb = pp.tile([128, CC, T], BF16)     # q^T, head cols on partitions
            kT_sb = pp.tile([128, CC, T], BF16)
            v_sb = pp.tile([128, TC, HEADS_PER_CORE, HD + 1], BF16)  # v | ones
            attnT_sb = pp.tile([128, CC, T], F32R)  # attention out, [cols, T]
            wo_sb = pp.tile([128, CC, D], F32R)
            bq_sb = pp.tile([128, CC], F32)
            bk_sb = pp.tile([128, CC], F32)
            bv_sb = pp.tile([128, CPC], F32)
            maskf = pp.tile([128, 128], F32)
            mask01 = pp.tile([128, 128], BF16)
            onesf = pp.tile([128, HD], F32)
            ones64 = pp.tile([128, HD], FP16)  # lhsT for denominator broadcast

            nc.sync.dma_start(wo_sb[:], wo.ap().rearrange("(c p) o -> p c o", p=128))
            nc.sync.dma_start(bq_sb[:], bq.ap())
            nc.sync.dma_start(bk_sb[:], bk.ap())
            nc.sync.dma_start(bv_sb[:], bv.ap())
            # mask01[j, i] = 1.0 if j <= i else 0.0 (valid causal region, S^T coords)
            make_upper_triangular(nc, maskf, val=1.0, diag=True)
            nc.vector.tensor_copy(mask01[:], maskf[:])
            nc.gpsimd.memset(v_sb[:, :, :, HD : HD + 1], 1.0)
            nc.gpsimd.memset(onesf[:], 1.0)
            nc.vector.tensor_copy(ones64[:], onesf[:])

            # ---------------- Phase A: qkv projection ----------------
            with (
                tc.tile_pool(name="loadA", bufs=1) as pA,
                tc.tile_pool(name="psumA", bufs=3, space="PSUM") as psA,
            ):
                xT_sb = pA.tile([128, DC, T], F32R)
                wq_sb = pA.tile([128, DC, CPC], F32R)
                wk_sb = pA.tile([128, DC, CPC], F32R)
                wv_sb = pA.tile([128, DC, CPC], F32R)
                xT_r = xT.ap().rearrange("(o p) t -> p o t", p=128)
                wq_r = wq.ap().rearrange("(o p) c -> p o c", p=128)
                wk_r = wk.ap().rearrange("(o p) c -> p o c", p=128)
                wv_r = wv.ap().rearrange("(o p) c -> p o c", p=128)
                for di in range(DC):
                    nc.sync.dma_start(wv_sb[:, di], wv_r[:, di])
                    nc.sync.dma_start(xT_sb[:, di], xT_r[:, di])
                    nc.sync.dma_start(wq_sb[:, di], wq_r[:, di])
                    nc.sync.dma_start(wk_sb[:, di], wk_r[:, di])

                for tj in range(TC):
                    ps_v = psA.tile([128, CPC], F32, tag="ps_v")
                    for di in range(DC):
                        nc.tensor.matmul(
                            ps_v[:],
                            xT_sb[:, di, tj * 128 : (tj + 1) * 128],
                            wv_sb[:, di, :],
                            start=(di == 0),
                            stop=(di == DC - 1),
                        )
                    nc.vector.tensor_add(v_sb[:, tj, :, :HD], ps_v[:], bv_sb[:])

                for dst, w_sb, b_sb in ((qT_sb, wq_sb, bq_sb), (kT_sb, wk_sb, bk_sb)):
                    for hc in range(CC):
                        for tj in range(TW):
                            ps_proj = psA.tile([128, WT], F32, tag="ps_proj")
                            for di in range(DC):
                                nc.tensor.matmul(
                                    ps_proj[:],
                                    w_sb[:, di, hc * 128 : (hc + 1) * 128],
                                    xT_sb[:, di, tj * WT : (tj + 1) * WT],
                                    start=(di == 0),
                                    stop=(di == DC - 1),
                                )
                            nc.vector.tensor_scalar_add(
                                dst[:, hc, tj * WT : (tj + 1) * WT],
                                ps_proj[:],
                                b_sb[:, hc : hc + 1],
                            )

            # ---------- Phase B: causal attention, big-N formulation ----------
            with (
                tc.tile_pool(name="pB", bufs=2) as pB,
                tc.tile_pool(name="outp", bufs=3) as outp,
                tc.tile_pool(name="psumB", bufs=3, space="PSUM") as psB,
            ):
                def emit_scores(h):
                    """S^T + exp + diag mask for one head into a ragged pT tile."""
                    hc = h // 2
                    po = (h % 2) * HD
                    pT = pB.tile([128, PT_COLS], BF16, tag="pT", name=f"pT{h}")
                    for jc in range(TC):
                        w_cols = (TC - jc) * 128
                        lhsT = kT_sb[po : po + HD, hc, jc * 128 : (jc + 1) * 128]
                        for s0 in range(0, w_cols, 2 * WT):
                            ssz = min(2 * WT, w_cols - s0)
                            st = psB.tile([128, 2 * WT], F32, tag="st", bufs=2)
                            for c0 in range(0, ssz, WT):
                                csz = min(WT, ssz - c0)
                                nc.tensor.matmul(
                                    st[:, c0 : c0 + csz],
                                    lhsT,
                                    qT_sb[
                                        po : po + HD,
                                        hc,
                                        jc * 128 + s0 + c0 : jc * 128 + s0 + c0 + csz,
                                    ],
                                    start=True,
                                    stop=True,
                                )
                            nc.scalar.activation(
                                pT[:, OFF[jc] + s0 : OFF[jc] + s0 + ssz],
                                st[:, :ssz],
                                mybir.ActivationFunctionType.Exp,
                            )
                        nc.vector.tensor_mul(
                            pT[:, OFF[jc] : OFF[jc] + 128],
                            pT[:, OFF[jc] : OFF[jc] + 128],
                            mask01[:],
                        )
                    return pT

                def emit_div(h, q, oT):
                    """Normalize quarter q: 1/l via exp(-ln(l)) on ACT rows,
                    PE K=1 matmul broadcasts it to 64 partitions, DVE scales."""
                    hc = h // 2
                    odd = h % 2 == 1
                    i0 = q * WT
                    lr = pB.tile([128, WT], FP16, tag="lr", name=f"lr{h}_{q}")
                    nc.scalar.copy(lr[HD : HD + 1, :], oT[HD : HD + 1, :])
                    rcr = pB.tile([128, WT], FP16, tag="rcr", name=f"rcr{h}_{q}")
                    with nc.allow_low_precision(reason="softmax denom recip in fp16"):
                        nc.vector.reciprocal(rcr[HD : HD + 1, :], lr[HD : HD + 1, :])
                    lP = psB.tile([128, WT], F32, tag="lP", bufs=1)
                    nc.tensor.matmul(
                        lP[0:HD, :],
                        ones64[HD : HD + 1, :],
                        rcr[HD : HD + 1, :],
                        start=True,
                        stop=True,
                    )
                    rcb = pB.tile([128, WT], F32, tag="rcb", name=f"rcb{h}_{q}")
                    nc.vector.tensor_copy(rcb[:HD, :], lP[:HD, :])
                    if not odd:
                        nc.vector.tensor_mul(
                            attnT_sb[:HD, hc, i0 : i0 + WT], oT[:HD, :], rcb[:HD, :]
                        )
                    else:
                        tmp = pB.tile([HD, WT], F32R, tag="pvtmp", name=f"pvt{h}_{q}")
                        nc.vector.tensor_mul(tmp[:], oT[:HD, :], rcb[:HD, :])
                        nc.sync.dma_start(attnT_sb[HD:, hc, i0 : i0 + WT], tmp[:])

                def emit_pv(h, pT):
                    """PV in transposed form: attnT[head rows, :] = (P V)^T / l.
                    Divides lag one quarter to stay off the PE critical path."""
                    oTs = []
                    for q in range(TW):
                        i0 = q * WT
                        jhi = min(4 * q + 3, TC - 1)
                        oT = psB.tile([128, WT], F32, tag="oT", bufs=3, name=f"oT{h}_{q}")
                        oTs.append(oT)
                        for jc in range(jhi + 1):
                            lo = max(jc * 128, i0)
                            rhs = pT[
                                :, OFF[jc] + lo - jc * 128 : OFF[jc] + i0 + WT - jc * 128
                            ]
                            nc.tensor.matmul(
                                oT[: HD + 1, lo - i0 : WT],
                                v_sb[:, jc, h, :],
                                rhs,
                                start=(jc == 0),
                                stop=(jc == jhi),
                            )
                        if q >= 1:
                            emit_div(h, q - 1, oTs[q - 1])
                    emit_div(h, TW - 1, oTs[TW - 1])

                prev = emit_scores(0)
                for h in range(1, HEADS_PER_CORE):
                    cur = emit_scores(h)
                    emit_pv(h - 1, prev)
                    prev = cur
                emit_pv(HEADS_PER_CORE - 1, prev)

                # ---------------- Phase D: W_o ----------------
                for tj in range(TW):
                    for oc in range(OC):
                        ps_wo = psB.tile([128, WT], F32, tag="oT", bufs=3)
                        for dc in range(CC):
                            nc.tensor.matmul(
                                ps_wo[:],
                                wo_sb[:, dc, oc * 128 : (oc + 1) * 128],
                                attnT_sb[:, dc, tj * WT : (tj + 1) * WT],
                                start=(dc == 0),
                                stop=(dc == CC - 1),
                            )
                        ot = outp.tile([128, WT], F32, tag="ot")
                        nc.vector.tensor_copy(ot[:], ps_wo[:])
                        nc.sync.dma_start(
                            yT.ap()[oc * 128 : (oc + 1) * 128, tj * WT : (tj + 1) * WT],
                            ot[:],
                        )

    nc.compile()
    return nc


def _get_nc():
    if "nc" not in _CACHE:
        _CACHE["nc"] = _build_nc()
    return _CACHE["nc"]


def kernel(x, W_qkv, b_qkv, W_o, b_o, **run_kwargs):
    x = np.asarray(x, dtype=np.float32)
    W_qkv = np.asarray(W_qkv, dtype=np.float32)
    b_qkv = np.asarray(b_qkv, dtype=np.float32)
    W_o = np.asarray(W_o, dtype=np.float32)
    b_o = np.asarray(b_o, dtype=np.float32)

    scale = np.float32(1.0) / np.sqrt(np.float32(HD)).astype(np.float32)

    in_maps = []
    for c in range(N_CORES):
        b = c // 2
        g = c % 2
        cs = g * CPC
        q_sl = slice(cs, cs + CPC)
        k_sl = slice(D + cs, D + cs + CPC)
        v_sl = slice(2 * D + cs, 2 * D + cs + CPC)
        in_maps.append(
            {
                "xT": np.ascontiguousarray(x[b].T),
                "wq": np.ascontiguousarray(W_qkv[:, q_sl]) * scale,
                "wk": np.ascontiguousarray(W_qkv[:, k_sl]),
                "wv": np.ascontiguousarray(W_qkv[:, v_sl]),
                "bq": np.ascontiguousarray((b_qkv[q_sl] * scale).reshape(CC, 128).T),
                "bk": np.ascontiguousarray(b_qkv[k_sl].reshape(CC, 128).T),
                "bv": np.ascontiguousarray(np.broadcast_to(b_qkv[v_sl], (128, CPC))),
                "wo": np.ascontiguousarray(W_o[cs : cs + CPC, :]),
            }
        )

    nc = _get_nc()
    res = run_bass_kernel_spmd(nc, in_maps, core_ids=list(range(N_CORES)), **run_kwargs)
    _CACHE["last_result"] = res

    out = np.empty((B, T, D), dtype=np.float32)
    for b in range(B):
        acc = res.results[2 * b]["yT"] + res.results[2 * b + 1]["yT"]
        out[b] = acc.T + b_o
    return out


# revision 28
# speedup vs baseline: 1.0132x; 1.0132x over previous
import sys

if "/opt/trn_rl_repo" not in sys.path:
    sys.path.insert(0, "/opt/trn_rl_repo")

import numpy as np

import concourse.bass as bass
import concourse.tile as tile
from concourse import bacc, mybir
from concourse.bass_utils import run_bass_kernel_spmd
from concourse.masks import make_identity, make_upper_triangular

F32 = mybir.dt.float32
F32R = mybir.dt.float32r
BF16 = mybir.dt.bfloat16
FP16 = mybir.dt.float16

# Problem shape (hardcoded per contract)
B, T, D = 4, 2048, 768
H, HD = 12, 64
N_CORES = 8
HEADS_PER_CORE = 6          # 12 heads / 2 groups
CPC = HEADS_PER_CORE * HD   # 384 qkv columns per core
TC = T // 128               # 16 token tiles of 128
DC = D // 128               # 6 chunks of the model dim
CC = CPC // 128             # 3 chunks of this core's head cols
OC = D // 128               # 6 output-col chunks
WT = 512                    # wide tile / matmul free-dim limit
TW = T // WT                # 4 wide token tiles

# ragged P^T layout: block jc covers i in [jc*128, T); OFF[jc] is its col offset
OFF = [0] * (TC + 1)
for _jc in range(TC):
    OFF[_jc + 1] = OFF[_jc] + (TC - _jc) * 128
PT_COLS = OFF[TC]  # 17408

_CACHE = {}


def _build_nc():
    nc = bacc.Bacc("TRN2", target_bir_lowering=False, debug=False)

    xT = nc.dram_tensor("xT", [D, T], F32R, kind="ExternalInput")
    wq = nc.dram_tensor("wq", [D, CPC], F32R, kind="ExternalInput")
    wk = nc.dram_tensor("wk", [D, CPC], F32R, kind="ExternalInput")
    wv = nc.dram_tensor("wv", [D, CPC], F32R, kind="ExternalInput")
    bq = nc.dram_tensor("bq", [128, CC], F32, kind="ExternalInput")
    bk = nc.dram_tensor("bk", [128, CC], F32, kind="ExternalInput")
    bv = nc.dram_tensor("bv", [128, CPC], F32, kind="ExternalInput")
    wo = nc.dram_tensor("wo", [CPC, D], F32R, kind="ExternalInput")
    yT = nc.dram_tensor("yT", [D, T], F32, kind="ExternalOutput")

    with tile.TileContext(nc) as tc:
        with tc.tile_pool(name="persist", bufs=1) as pp:
            qT_sb = pp.tile([128, CC, T], BF16)     # q^T, head cols on partitions
            kT_sb = pp.tile([128, CC, T], BF16)
            v_sb = pp.tile([128, TC, HEADS_PER_CORE, HD + 1], BF16)  # v | ones
            attnT_sb = pp.tile([128, CC, T], F32R)  # attention out, [cols, T]
            wo_sb = pp.tile([128, CC, D], F32R)
            bq_sb = pp.tile([128, CC], F32)
            bk_sb = pp.tile([128, CC], F32)
            bv_sb = pp.tile([128, CPC], F32)
            maskf = pp.tile([128, 128], F32)
            mask01 = pp.tile([128, 128], BF16)
            onesf = pp.tile([128, HD], F32)
            ones64 = pp.tile([128, HD], FP16)  # lhsT for denominator broadcast

            nc.sync.dma_start(wo_sb[:], wo.ap().rearrange("(c p) o -> p c o", p=128))
            nc.sync.dma_start(bq_sb[:], bq.ap())
            nc.sync.dma_start(bk_sb[:], bk.ap())
            nc.sync.dma_start(bv_sb[:], bv.ap())
            # mask01[j, i] = 1.0 if j <= i else 0.0 (valid causal region, S^T coords)
            make_upper_triangular(nc, maskf, val=1.0, diag=True)
            nc.vector.tensor_copy(mask01[:], maskf[:])
            nc.gpsimd.memset(v_sb[:, :, :, HD : HD + 1], 1.0)
            nc.gpsimd.memset(onesf[:], 1.0)
            nc.vector.tensor_copy(ones64[:], onesf[:])

            # ---------------- Phase A: qkv projection ----------------
            with (
                tc.tile_pool(name="loadA", bufs=1) as pA,
                tc.tile_pool(name="psumA", bufs=3, space="PSUM") as psA,
            ):
                xT_sb = pA.tile([128, DC, T], F32R)
                wq_sb = pA.tile([128, DC, CPC], F32R)
                wk_sb = pA.tile([128, DC, CPC], F32R)
                wv_sb = pA.tile([128, DC, CPC], F32R)
                xT_r = xT.ap().rearrange("(o p) t -> p o t", p=128)
                wq_r = wq.ap().rearrange("(o p) c -> p o c", p=128)
                wk_r = wk.ap().rearrange("(o p) c -> p o c", p=128)
                wv_r = wv.ap().rearrange("(o p) c -> p o c", p=128)
                for di in range(DC):
                    nc.sync.dma_start(wv_sb[:, di], wv_r[:, di])
                    nc.sync.dma_start(xT_sb[:, di], xT_r[:, di])
                    nc.sync.dma_start(wq_sb[:, di], wq_r[:, di])
                    nc.sync.dma_start(wk_sb[:, di], wk_r[:, di])

                for tj in range(TC):
                    ps_v = psA.tile([128, CPC], F32, tag="ps_v")
                    for di in range(DC):
                        nc.tensor.matmul(
                            ps_v[:],
                            xT_sb[:, di, tj * 128 : (tj + 1) * 128],
                            wv_sb[:, di, :],
                            start=(di == 0),
                            stop=(di == DC - 1),
                        )
                    nc.vector.tensor_add(v_sb[:, tj, :, :HD], ps_v[:], bv_sb[:])

                for dst, w_sb, b_sb in ((qT_sb, wq_sb, bq_sb), (kT_sb, wk_sb, bk_sb)):
                    for hc in range(CC):
                        for tj in range(TW):
                            ps_proj = psA.tile([128, WT], F32, tag="ps_proj")
                            for di in range(DC):
                                nc.tensor.matmul(
                                    ps_proj[:],
                                    w_sb[:, di, hc * 128 : (hc + 1) * 128],
                                    xT_sb[:, di, tj * WT : (tj + 1) * WT],
                                    start=(di == 0),
                                    stop=(di == DC - 1),
                                )
                            nc.vector.tensor_scalar_add(
                                dst[:, hc, tj * WT : (tj + 1) * WT],
                                ps_proj[:],
                                b_sb[:, hc : hc + 1],
                            )

            # ---------- Phase B: causal attention, big-N formulation ----------
            with (
                tc.tile_pool(name="pB", bufs=2) as pB,
                tc.tile_pool(name="outp", bufs=3) as outp,
                tc.tile_pool(name="psumB", bufs=3, space="PSUM") as psB,
            ):
                def score_steps(h, pT):
                    """Generate per-slot closures: S^T matmuls + exp (+ mask)."""
                    hc = h // 2
                    po = (h % 2) * HD
                    for jc in range(TC):
                        w_cols = (TC - jc) * 128
                        lhsT = kT_sb[po : po + HD, hc, jc * 128 : (jc + 1) * 128]
                        for s0 in range(0, w_cols, 2 * WT):
                            ssz = min(2 * WT, w_cols - s0)

                            def step(jc=jc, s0=s0, ssz=ssz, lhsT=lhsT):
                                st = psB.tile([128, 2 * WT], F32, tag="st", bufs=2)
                                for c0 in range(0, ssz, WT):
                                    csz = min(WT, ssz - c0)
                                    nc.tensor.matmul(
                                        st[:, c0 : c0 + csz],
                                        lhsT,
                                        qT_sb[
                                            po : po + HD,
                                            hc,
                                            jc * 128 + s0 + c0 : jc * 128
                                            + s0
                                            + c0
                                            + csz,
                                        ],
                                        start=True,
                                        stop=True,
                                    )
                                nc.scalar.activation(
                                    pT[:, OFF[jc] + s0 : OFF[jc] + s0 + ssz],
                                    st[:, :ssz],
                                    mybir.ActivationFunctionType.Exp,
                                )
                                if s0 == 0:
                                    nc.vector.tensor_mul(
                                        pT[:, OFF[jc] : OFF[jc] + 128],
                                        pT[:, OFF[jc] : OFF[jc] + 128],
                                        mask01[:],
                                    )

                            yield step

                def emit_div(h, q, oT):
                    """Normalize quarter q: 1/l via exp(-ln(l)) on ACT rows,
                    PE K=1 matmul broadcasts it to 64 partitions, DVE scales."""
                    hc = h // 2
                    odd = h % 2 == 1
                    i0 = q * WT
                    lr = pB.tile([128, WT], FP16, tag="lr", name=f"lr{h}_{q}")
                    nc.scalar.copy(lr[HD : HD + 1, :], oT[HD : HD + 1, :])
                    rcr = pB.tile([128, WT], FP16, tag="rcr", name=f"rcr{h}_{q}")
                    with nc.allow_low_precision(reason="softmax denom recip in fp16"):
                        nc.vector.reciprocal(rcr[HD : HD + 1, :], lr[HD : HD + 1, :])
                    lP = psB.tile([128, WT], F32, tag="lP", bufs=1)
                    nc.tensor.matmul(
                        lP[0:HD, :],
                        ones64[HD : HD + 1, :],
                        rcr[HD : HD + 1, :],
                        start=True,
                        stop=True,
                    )
                    rcb = pB.tile([128, WT], F32, tag="rcb", name=f"rcb{h}_{q}")
                    nc.vector.tensor_copy(rcb[:HD, :], lP[:HD, :])
                    if not odd:
                        nc.vector.tensor_mul(
                            attnT_sb[:HD, hc, i0 : i0 + WT], oT[:HD, :], rcb[:HD, :]
                        )
                    else:
                        tmp = pB.tile([HD, WT], F32R, tag="pvtmp", name=f"pvt{h}_{q}")
                        nc.vector.tensor_mul(tmp[:], oT[:HD, :], rcb[:HD, :])
                        nc.sync.dma_start(attnT_sb[HD:, hc, i0 : i0 + WT], tmp[:])

                def d_steps(tj):
                    """W_o matmuls for one completed 512-wide token block."""
                    for oc in range(OC):

                        def step(oc=oc):
                            ps_wo = psB.tile([128, WT], F32, tag="oT", bufs=3)
                            for dc in range(CC):
                                nc.tensor.matmul(
                                    ps_wo[:],
                                    wo_sb[:, dc, oc * 128 : (oc + 1) * 128],
                                    attnT_sb[:, dc, tj * WT : (tj + 1) * WT],
                                    start=(dc == 0),
                                    stop=(dc == CC - 1),
                                )
                            ot = outp.tile([128, WT], F32, tag="ot")
                            nc.vector.tensor_copy(ot[:], ps_wo[:])
                            nc.sync.dma_start(
                                yT.ap()[
                                    oc * 128 : (oc + 1) * 128, tj * WT : (tj + 1) * WT
                                ],
                                ot[:],
                            )

                        yield step

                def pv_steps(h, pT, d_queue):
                    """PV in transposed form (attnT = (P V)^T / l), as steps.
                    Divides lag one quarter; the last head enqueues W_o work."""
                    oTs = {}
                    for q in range(TW):
                        i0 = q * WT
                        jhi = min(4 * q + 3, TC - 1)
                        jcs = list(range(jhi + 1))
                        # chunk the accumulation into groups of <=6 matmuls
                        for g0 in range(0, len(jcs), 6):
                            grp = jcs[g0 : g0 + 6]

                            def step(q=q, i0=i0, jhi=jhi, grp=grp, g0=g0):
                                if g0 == 0:
                                    oTs[q] = psB.tile(
                                        [128, WT], F32, tag="oT", bufs=3,
                                        name=f"oT{h}_{q}",
                                    )
                                oT = oTs[q]
                                for jc in grp:
                                    lo = max(jc * 128, i0)
                                    rhs = pT[
                                        :,
                                        OFF[jc] + lo - jc * 128 : OFF[jc]
                                        + i0
                                        + WT
                                        - jc * 128,
                                    ]
                                    nc.tensor.matmul(
                                        oT[: HD + 1, lo - i0 : WT],
                                        v_sb[:, jc, h, :],
                                        rhs,
                                        start=(jc == 0),
                                        stop=(jc == jhi),
                                    )

                            yield step
                        if q >= 1:
                            yield lambda q=q: emit_div(h, q - 1, oTs[q - 1])
                            if d_queue is not None:
                                yield from d_steps(q - 1)
                    yield lambda: emit_div(h, TW - 1, oTs[TW - 1])
                    if d_queue is not None:
                        yield from d_steps(TW - 1)

                def interleave(a_steps, b_steps):
                    """Emit steps from both lists, spreading b evenly among a."""
                    a, b = list(a_steps), list(b_steps)
                    if not b:
                        for s in a:
                            s()
                        return
                    ratio = max(1, len(a) // len(b))
                    bi = 0
                    for idx, s in enumerate(a):
                        s()
                        if idx % ratio == ratio - 1 and bi < len(b):
                            b[bi]()
                            bi += 1
                    while bi < len(b):
                        b[bi]()
                        bi += 1

                pTs = {}
                pTs[0] = pB.tile([128, PT_COLS], BF16, tag="pT", name="pT0")
                for s in score_steps(0, pTs[0]):
                    s()
                for h in range(1, HEADS_PER_CORE):
                    pTs[h] = pB.tile([128, PT_COLS], BF16, tag="pT", name=f"pT{h}")
                    interleave(
                        score_steps(h, pTs[h]),
                        pv_steps(h - 1, pTs[h - 1], None),
                    )
                    del pTs[h - 1]
                last = HEADS_PER_CORE - 1
                for s in pv_steps(last, pTs[last], d_queue=True):
                    s()

    nc.compile()
    return nc


def _get_nc():
    if "nc" not in _CACHE:
        _CACHE["nc"] = _build_nc()
    return _CACHE["nc"]


def kernel(x, W_qkv, b_qkv, W_o, b_o, **run_kwargs):
    x = np.asarray(x, dtype=np.float32)
    W_qkv = np.asarray(W_qkv, dtype=np.float32)
    b_qkv = np.asarray(b_qkv, dtype=np.float32)
    W_o = np.asarray(W_o, dtype=np.float32)
    b_o = np.asarray(b_o, dtype=np.float32)

    scale = np.float32(1.0) / np.sqrt(np.float32(HD)).astype(np.float32)

    in_maps = []
    for c in range(N_CORES):
        b = c // 2
        g = c % 2
        cs = g * CPC
        q_sl = slice(cs, cs + CPC)
        k_sl = slice(D + cs, D + cs + CPC)
        v_sl = slice(2 * D + cs, 2 * D + cs + CPC)
        in_maps.append(
            {
                "xT": np.ascontiguousarray(x[b].T),
                "wq": np.ascontiguousarray(W_qkv[:, q_sl]) * scale,
                "wk": np.ascontiguousarray(W_qkv[:, k_sl]),
                "wv": np.ascontiguousarray(W_qkv[:, v_sl]),
                "bq": np.ascontiguousarray((b_qkv[q_sl] * scale).reshape(CC, 128).T),
                "bk": np.ascontiguousarray(b_qkv[k_sl].reshape(CC, 128).T),
                "bv": np.ascontiguousarray(np.broadcast_to(b_qkv[v_sl], (128, CPC))),
                "wo": np.ascontiguousarray(W_o[cs : cs + CPC, :]),
            }
        )

    nc = _get_nc()
    res = run_bass_kernel_spmd(nc, in_maps, core_ids=list(range(N_CORES)), **run_kwargs)
    _CACHE["last_result"] = res

    out = np.empty((B, T, D), dtype=np.float32)
    for b in range(B):
        acc = res.results[2 * b]["yT"] + res.results[2 * b + 1]["yT"]
        out[b] = acc.T + b_o
    return out
